# revision 11
# baseline (speedup 1.0000x reference)
"""Trainium2 Bass kernel for nn_BraskModel (nms_detection).

Strategy
--------
The reference builds a (B,R,K,L,H) broadcast-fused tensor and applies 1-wide
linear sigmoid heads to it.  Because the heads are linear, the huge tensor
decomposes exactly:

    fts[b,r,k,l] = sigmoid( t1[b,k] + t2[b,l] + t3[b,r] + const )

with
    t1[b,k] = m[b,k] * (sk[b,k,:] @ (ws @ u) + bs @ u)        (span terms)
    t2[b,l] = X[b,l,:] @ (wx @ u + u) + bx @ u                (per-token)
    t3[b,r] = sum_l a[b,r,l] * (X[b,l,:] @ u)                 (attention)

so only the attention scores e[b,r,l] = sum_a tanh(wxa[b,l,a]+wr[r,a]+wg[b,a])*v[a]
involve heavy compute.  vb cancels in softmax.

Sharding: 8 cores = 2 groups of 4.  Cores 0-3 run the semantic attention /
forward path (fts,fte), cores 4-7 the transe attention / backward path
(bhs,bhe); within a group each core owns R/4 = 4 relations.

Perf notes: fp32 matmul streams at 4 cycles/row on TRN2; float32r streams at
1 cycle/row for N>=256, costing ~1.4e-4 relative matmul error (~13-bit
mantissa), which stays well inside tolerance after softmax amplification.
All heavy matmuls run f32r.  Small constants are packed into three blobs so
the DMA ring isn't serialized by ~25 tiny descriptors before the big loads.

Host side does only: input marshalling (X transpose), tiny weight-derived
projections (rel @ wr etc., ~5M MACs), and the inherently sequential greedy
span matcher (384-step boolean scan, whose decision margins ~3e-5 are ~100x
above fp32 rounding noise).
"""

import numpy as np

import concourse.bacc as bacc
import concourse.bass as bass
import concourse.mybir as mybir
import concourse.tile as tile
from concourse.bass_utils import run_bass_kernel_spmd

B, L, H, R, RD, A = 2, 384, 768, 16, 256, 256
K_MAX, MAX_SPAN, THR = 8, 10, 0.5
N_CORES = 8
GROUP = 4          # cores per attention group
RLOC = R // GROUP  # relations per core
BL = B * L         # 768
F32 = mybir.dt.float32
F32R = mybir.dt.float32r
F16 = mybir.dt.float16
KC = H // 128      # 6 contraction chunks

# --- CONSTF (fp32, 128 x 266): col 0 t1f | col 1 pbh | 2:10 wr | 10:138 sels
#     | 138:266 sele
CF_T1F, CF_PBH, CF_WR, CF_SELS, CF_SELE, CF_COLS = 0, 1, 2, 10, 138, 266
# --- CONSTR (f32r, 128 x 176): 0:48 v12 chunks | 48:176 vmask chunks
CR_V12, CR_VM, CR_COLS = 0, 48, 176
# --- CONSTROW (f32r, 1 x 1424): 0:512 wg | 512:528 eb | 528:1040 ob | 1040:1424 ones
CW_WG, CW_EB, CW_OB, CW_ONES, CW_COLS = 0, 512, 528, 1040, 1424

_compiled = {}


# ----------------------------------------------------------------------------
# Bass program (identical on all 8 cores; per-core behaviour is data-driven)
# ----------------------------------------------------------------------------
def _build_nc(debug=False):
    nc = bacc.Bacc("TRN2", target_bir_lowering=False, debug=debug, num_devices=1)

    xt_d = nc.dram_tensor("xt", (H, BL), F16, kind="ExternalInput")
    wx_d = nc.dram_tensor("wxm", (H, A), F16, kind="ExternalInput")
    cr_d = nc.dram_tensor("constr", (128, CR_COLS), F16, kind="ExternalInput")
    cw_d = nc.dram_tensor("constrow", (1, CW_COLS), F32R, kind="ExternalInput")
    cf_d = nc.dram_tensor("constf", (128, CF_COLS), F32, kind="ExternalInput")

    big_d = nc.dram_tensor("big", (128, 384), F32, kind="ExternalOutput")
    probs_d = nc.dram_tensor("probs", (4, BL), F32, kind="ExternalOutput")

    with tile.TileContext(nc) as tc:
        with (
            tc.tile_pool(name="const", bufs=1) as cpool,
            tc.tile_pool(name="work", bufs=1) as wpool,
            tc.tile_pool(name="zpool", bufs=3) as zpool,
            tc.tile_pool(name="ps_big", bufs=4, space="PSUM") as ps_big,
            tc.tile_pool(name="ps_small", bufs=2, space="PSUM") as ps_small,
        ):
            # ---- loads ------------------------------------------------------
            # Each dma_start costs ~0.6us of serial descriptor time on its
            # ring, so batch: XT lands as one (128, 6*768) tile via two large
            # DMAs (3 contraction chunks each, so matmuls start after half),
            # WX as one (128, 6*256) tile, spread over the sync / scalar /
            # gpsimd rings.
            xtb = cpool.tile([128, KC * BL], F16, tag="xtb")
            wxb = cpool.tile([128, KC * A], F16, tag="wxb")
            cr_sb = cpool.tile([128, CR_COLS], F16, tag="cr")
            cw_sb = cpool.tile([1, CW_COLS], F32R, tag="cw")
            cf_sb = cpool.tile([128, CF_COLS], F32, tag="cf")
            xt_r = xt_d[:].rearrange("(c p) l -> p c l", p=128)
            wx_r = wx_d[:].rearrange("(c p) a -> p c a", p=128)
            nc.sync.dma_start(wxb[:], wx_r)
            nc.sync.dma_start(xtb[:, 0:3 * BL], xt_r[:, 0:3, :])
            nc.scalar.dma_start(xtb[:, 3 * BL:KC * BL], xt_r[:, 3:KC, :])
            nc.gpsimd.dma_start(cr_sb[:], cr_d[:])
            nc.gpsimd.dma_start(cw_sb[:], cw_d[:])
            nc.sync.dma_start(cf_sb[:], cf_d[:])

            def xt_sl(k, lo, hi):
                return xtb[:, k * BL + lo: k * BL + hi]

            def wx_sl(k, at):
                return wxb[:, k * A + at * 128: k * A + (at + 1) * 128]

            def v12(k):
                return cr_sb[:, CR_V12 + 8 * k: CR_V12 + 8 * (k + 1)]

            def vmask(c, p):
                return cr_sb[:, CR_VM + 64 * c + 8 * p: CR_VM + 64 * c + 8 * (p + 1)]

            def wg_row(b, at):
                o = CW_WG + 256 * b + 128 * at
                return cw_sb[0:1, o:o + 128]

            def eb_row(b):
                return cw_sb[0:1, CW_EB + 8 * b: CW_EB + 8 * (b + 1)]

            def ob_row(v):
                return cw_sb[0:1, CW_OB + 128 * v: CW_OB + 128 * (v + 1)]

            ones384 = cw_sb[0:1, CW_ONES:CW_ONES + 384]

            # ---- XS = V12^T @ X^T -> (8, BL) and WXA = WX^T @ X^T (+wg) ----
            xs_ps = [ps_small.tile([8, 384], F32, tag="ps_small", name=f"xs_ps{i}")
                     for i in range(2)]
            wxa_ps = [ps_big.tile([128, 384], F32, tag="ps_big", name=f"wxa_ps{i}")
                      for i in range(4)]
            for k in range(KC):
                for at in range(2):
                    for b in range(2):
                        nc.tensor.matmul(
                            wxa_ps[at * 2 + b][:], wx_sl(k, at),
                            xt_sl(k, b * 384, (b + 1) * 384),
                            start=(k == 0), stop=False,
                        )
                for half in range(2):
                    nc.tensor.matmul(
                        xs_ps[half][:], v12(k),
                        xt_sl(k, half * 384, (half + 1) * 384),
                        start=(k == 0), stop=(k == KC - 1),
                    )
            # fold wg[b,:] into wxa (rank-1 update): removes b from tanh bias
            for at in range(2):
                for b in range(2):
                    nc.tensor.matmul(
                        wxa_ps[at * 2 + b][:], wg_row(b, at), ones384,
                        start=False, stop=True,
                    )

            xs_sb = wpool.tile([8, BL], F32, tag="xs_sb")
            for half in range(2):
                nc.vector.tensor_copy(
                    xs_sb[:, half * 384:(half + 1) * 384], xs_ps[half][:])
            # xs_s, xs_e, t2_s, t2_e rows at base partition 0, rounded to f32r
            xsrow = [wpool.tile([1, BL], F32R, tag=f"xsrow_{i}", name=f"xsrow{i}")
                     for i in range(4)]
            for i in range(4):
                nc.gpsimd.dma_start(xsrow[i][:], xs_sb[4 + i:5 + i, :])
            wxa_sb = [wpool.tile([128, BL], F32, tag=f"wxa_sb{c}", name=f"wxa_sb{c}")
                      for c in range(2)]
            for at in range(2):
                for b in range(2):
                    nc.vector.tensor_copy(
                        wxa_sb[at][:, b * 384:(b + 1) * 384], wxa_ps[at * 2 + b][:])

            # ---- probs outputs: sigmoid(x) = 0.5*(1+tanh(x/2)) -------------
            pr_t = wpool.tile([4, BL], F32, tag="pr_t")
            nc.scalar.activation(pr_t[:], xs_sb[0:4, :],
                                 mybir.ActivationFunctionType.Tanh,
                                 bias=cf_sb[0:4, CF_PBH:CF_PBH + 1], scale=0.5)
            probs_sb = wpool.tile([4, BL], F32, tag="probs_sb")
            nc.vector.tensor_scalar(probs_sb[:], pr_t[:], 1.0, 0.5,
                                    mybir.AluOpType.add, mybir.AluOpType.mult)
            nc.sync.dma_start(probs_d[:], probs_sb[:])

            # ---- z = tanh(wxa + wr[rl]) ; E[pair,l] = sum_a z*v ------------
            e_ps = ps_small.tile([8, 384], F32, tag="ps_small", name="e_ps")
            n_emm = 0
            for rl in range(RLOC):
                for c in range(2):
                    z = zpool.tile([128, BL], F16, tag="z", name="z")
                    wrcol = CF_WR + 4 * c + rl
                    nc.scalar.activation(z[:], wxa_sb[c][:],
                                         mybir.ActivationFunctionType.Tanh,
                                         bias=cf_sb[:, wrcol:wrcol + 1], scale=1.0)
                    for b in range(2):
                        p = b * RLOC + rl
                        nc.tensor.matmul(
                            e_ps[:], vmask(c, p), z[:, b * 384:(b + 1) * 384],
                            start=(n_emm == 0), stop=(n_emm == 2 * 2 * RLOC - 1),
                        )
                        n_emm += 1

            # ---- softmax over l (per pair row) -----------------------------
            negmax = wpool.tile([8, 1], F32, tag="negmax")
            nc.vector.reduce_max(negmax[:], e_ps[:], axis=mybir.AxisListType.X,
                                 negate=True)
            exp_sb = wpool.tile([8, 384], F32, tag="exp_sb")
            esum = wpool.tile([8, 1], F32, tag="esum")
            nc.scalar.activation(exp_sb[:], e_ps[:],
                                 mybir.ActivationFunctionType.Exp,
                                 bias=negmax[:, 0:1], scale=1.0,
                                 accum_out=esum[:, 0:1])
            rsum = wpool.tile([8, 1], F32, tag="rsum")
            nc.vector.reciprocal(rsum[:], esum[:])

            # ---- t3[pair] = sum_l a * xs_u (per head) ----------------------
            t3 = [wpool.tile([8, 1], F32, tag=f"t3_{hu}", name=f"t3_{hu}")
                  for hu in range(2)]
            scr = wpool.tile([8, 384], F32, tag="scr")
            for hu in range(2):
                xb_ps = ps_small.tile([8, 384], F32, tag="ps_small", name="xb_ps")
                for b in range(2):
                    nc.tensor.matmul(
                        xb_ps[:], eb_row(b),
                        xsrow[hu][0:1, b * 384:(b + 1) * 384],
                        start=(b == 0), stop=(b == 1),
                    )
                nc.vector.tensor_mul(scr[:], exp_sb[:], xb_ps[:])
                t3u = wpool.tile([8, 1], F32, tag="t3u", name="t3u", bufs=2)
                nc.vector.reduce_sum(t3u[:], scr[:], axis=mybir.AxisListType.X)
                nc.vector.tensor_scalar_mul(t3[hu][:], t3u[:], rsum[:, 0:1])

            # ---- expand t3 to 128 output rows, add t1, halve ---------------
            t3e_ps = ps_big.tile([128, 1], F32, tag="ps_big", name="t3e_ps")
            nc.tensor.matmul(t3e_ps[:], cf_sb[0:8, CF_SELS:CF_SELS + 128],
                             t3[0][:], start=True, stop=False)
            nc.tensor.matmul(t3e_ps[:], cf_sb[0:8, CF_SELE:CF_SELE + 128],
                             t3[1][:], start=False, stop=True)
            bias_sb = wpool.tile([128, 1], F32, tag="bias_sb")
            nc.vector.tensor_scalar(bias_sb[:], t3e_ps[:],
                                    cf_sb[:, CF_T1F:CF_T1F + 1], 0.5,
                                    mybir.AluOpType.add, mybir.AluOpType.mult)

            # ---- big output: rows (head,b,rl,k) x 384 ----------------------
            tb_ps = ps_big.tile([128, 384], F32, tag="ps_big", name="tb_ps")
            n_tb = 0
            for hu in range(2):
                for b in range(2):
                    nc.tensor.matmul(
                        tb_ps[:], ob_row(hu * 2 + b),
                        xsrow[2 + hu][0:1, b * 384:(b + 1) * 384],
                        start=(n_tb == 0), stop=(n_tb == 3),
                    )
                    n_tb += 1
            th_sb = wpool.tile([128, 384], F32, tag="th_sb")
            nc.scalar.activation(th_sb[:], tb_ps[:],
                                 mybir.ActivationFunctionType.Tanh,
                                 bias=bias_sb[:, 0:1], scale=0.5)
            out_sb = wpool.tile([128, 384], F32, tag="out_sb")
            nc.vector.tensor_scalar(out_sb[:], th_sb[:], 1.0, 0.5,
                                    mybir.AluOpType.add, mybir.AluOpType.mult)
            nc.sync.dma_start(big_d[:], out_sb[:])

    nc.compile()
    return nc


# ----------------------------------------------------------------------------
# Host-side pieces
# ----------------------------------------------------------------------------
def _sigmoid(x):
    return 1.0 / (1.0 + np.exp(-x))


def _extract_sk(Xb, so, eo):
    """Greedy span matcher — exact transcription of reference.extract_sk."""
    Lp = L + MAX_SPAN
    eo_pad = np.concatenate([eo, np.zeros(MAX_SPAN, bool)])
    consumed = np.zeros(Lp, bool)
    valid = np.zeros(L, bool)
    e_arr = np.zeros(L, np.int64)
    for s in range(L):
        win = eo_pad[s:s + MAX_SPAN] & ~consumed[s:s + MAX_SPAN]
        ok = bool(so[s]) and bool(win.any())
        e = s + int(np.argmax(win))
        if ok:
            consumed[e] = True
        valid[s] = ok
        e_arr[s] = e
    rank = np.cumsum(valid) - 1
    slot = np.where(valid & (rank < K_MAX), rank, K_MAX)
    span = (Xb + Xb[np.clip(e_arr, 0, L - 1)]) * np.float32(0.5)
    sk = np.zeros((K_MAX + 1, H), np.float32)
    m = np.zeros((K_MAX + 1,), np.float32)
    np.add.at(sk, slot, np.where(valid[:, None], span, np.float32(0)))
    np.add.at(m, slot, valid.astype(np.float32))
    return sk[:K_MAX], m[:K_MAX]


def kernel(description_embeddings, description_mean_embeddings, description_ids,
           semantic_relation_embeddings, transe_relation_embeddings, params):
    p = {k: np.asarray(v, dtype=np.float32) for k, v in params.items()}
    X = np.asarray(description_embeddings, np.float32)
    Xm = np.asarray(description_mean_embeddings, np.float32)
    rels = {
        'f': np.asarray(semantic_relation_embeddings, np.float32),
        'b': np.asarray(transe_relation_embeddings, np.float32),
    }
    Xf = X.reshape(BL, H)
    XT = np.ascontiguousarray(Xf.T)
    XT16 = XT.astype(np.float16)

    # probabilities for the span matcher (margins >> fp rounding noise)
    pv = {}
    for pre in ('fhp', 'btp'):
        pv[pre + '_s'] = _sigmoid(Xf @ p[pre + '_sw'][:, 0] + p[pre + '_sb'][0])
        pv[pre + '_e'] = _sigmoid(Xf @ p[pre + '_ew'][:, 0] + p[pre + '_eb'][0])

    sk = {}
    msk = {}
    for path, pre in (('f', 'fhp'), ('b', 'btp')):
        so = (pv[pre + '_s'] >= THR).reshape(B, L)
        eo = (pv[pre + '_e'] >= THR).reshape(B, L)
        s_l, m_l = [], []
        for b in range(B):
            s_, m_ = _extract_sk(X[b], so[b], eo[b])
            s_l.append(s_)
            m_l.append(m_)
        sk[path] = np.stack(s_l)
        msk[path] = np.stack(m_l)

    pidx = np.arange(128)
    b_of_p = (pidx % 64) // 32
    rl_of_p = (pidx % 32) // 8
    k_of_p = pidx % 8

    # ---- CONSTROW blob (f32r semantics, fp32 storage); core-independent ----
    constrow = np.zeros((1, CW_COLS), np.float32)
    for b in range(B):
        constrow[0, CW_EB + 8 * b + b * RLOC: CW_EB + 8 * b + (b + 1) * RLOC] = 1.0
    for hu in range(2):
        for b in range(2):
            v0 = hu * 2 + b
            o = CW_OB + 128 * v0 + hu * 64 + b * 32
            constrow[0, o:o + 32] = 1.0
    constrow[0, CW_ONES:CW_ONES + 384] = 1.0

    # per-group (attention/path) data
    group = {}
    for path, att, fuse, head in (('f', 'sra', 'fef', 'ftp'),
                                  ('b', 'tra', 'feb', 'bhp')):
        rel = rels[path]
        wr_proj = rel @ p[att + '_wr'] + p[att + '_br']          # (R, A)
        wg_proj = Xm @ p[att + '_wg'] + p[att + '_bg']           # (B, A)
        v = p[att + '_v'][:, 0]                                  # (A,)

        cw = constrow.copy()
        cw[0, CW_WG:CW_WG + 512] = wg_proj.reshape(-1)

        constr = np.zeros((128, CR_COLS), np.float32)
        for k in range(KC):
            pass  # v12 filled below
        for c in range(2):
            for pp in range(8):
                constr[:, CR_VM + 64 * c + 8 * pp + pp] = v[c * 128:(c + 1) * 128]

        t1f = np.zeros((128,), np.float32)
        v12 = np.zeros((H, 8), np.float32)
        v12[:, 0] = p['fhp_sw'][:, 0]
        v12[:, 1] = p['fhp_ew'][:, 0]
        v12[:, 2] = p['btp_sw'][:, 0]
        v12[:, 3] = p['btp_ew'][:, 0]
        for hu, suf in enumerate(('s', 'e')):
            u = p[head + '_' + suf + 'w'][:, 0]
            ub = p[head + '_' + suf + 'b'][0]
            v12[:, 4 + hu] = u
            v12[:, 6 + hu] = p[fuse + '_wx'] @ u + u
            t1 = msk[path] * (sk[path] @ (p[fuse + '_ws'] @ u)
                              + p[fuse + '_bs'] @ u)             # (B, K)
            const = p[fuse + '_bx'] @ u + ub
            rows = slice(hu * 64, (hu + 1) * 64)
            t1f[rows] = t1[b_of_p[rows], k_of_p[rows]] + const
        for k in range(KC):
            constr[:, CR_V12 + 8 * k: CR_V12 + 8 * (k + 1)] = \
                v12[k * 128:(k + 1) * 128, :]

        group[path] = dict(
            wx16=np.ascontiguousarray(p[att + '_wx']).astype(np.float16), wr_proj=wr_proj,
            constr=constr, cw=cw, t1f=t1f,
        )

    # ---- CONSTF (per core: wr slice differs) -------------------------------
    sels = np.zeros((8, 128), np.float32)
    sele = np.zeros((8, 128), np.float32)
    for pp in range(128):
        q = b_of_p[pp] * RLOC + rl_of_p[pp]
        (sels if pp < 64 else sele)[q, pp] = 1.0
    pbh = 0.5 * np.array([p['fhp_sb'][0], p['fhp_eb'][0],
                          p['btp_sb'][0], p['btp_eb'][0]], np.float32)

    in_maps = []
    for core in range(N_CORES):
        path = 'f' if core < GROUP else 'b'
        g = core % GROUP
        gd = group[path]
        constf = np.zeros((128, CF_COLS), np.float32)
        constf[:, CF_T1F] = gd['t1f']
        constf[0:4, CF_PBH] = pbh
        for c in range(2):
            for rl in range(RLOC):
                constf[:, CF_WR + 4 * c + rl] = \
                    gd['wr_proj'][g * RLOC + rl, c * 128:(c + 1) * 128]
        constf[0:8, CF_SELS:CF_SELS + 128] = sels
        constf[0:8, CF_SELE:CF_SELE + 128] = sele
        in_maps.append({
            'xt': XT16, 'wxm': gd['wx16'], 'constr': gd['constr'].astype(np.float16),
            'constrow': gd['cw'], 'constf': constf,
        })

    if 'nc' not in _compiled:
        _compiled['nc'] = _build_nc(debug=False)
    nc = _compiled['nc']
    res = run_bass_kernel_spmd(nc, in_maps, core_ids=list(range(N_CORES)))
    outs = res.results

    probs = outs[0]['probs']
    fhs = probs[0].reshape(B, L, 1).astype(np.float32)
    fhe = probs[1].reshape(B, L, 1).astype(np.float32)
    bts = probs[2].reshape(B, L, 1).astype(np.float32)
    bte = probs[3].reshape(B, L, 1).astype(np.float32)

    def gather_big(cores, row0):
        full = np.zeros((B, R, K_MAX, L, 1), np.float32)
        for g, core in enumerate(cores):
            blk = outs[core]['big'][row0:row0 + 64].reshape(2, RLOC, K_MAX, L)
            full[:, g * RLOC:(g + 1) * RLOC] = blk[..., None]
        return full

    fts = gather_big(range(0, 4), 0)
    fte = gather_big(range(0, 4), 64)
    bhs = gather_big(range(4, 8), 0)
    bhe = gather_big(range(4, 8), 64)

    return fhs, fhe, fts, fte, bts, bte, bhs, bhe


# revision 12
# speedup vs baseline: 1.0253x; 1.0253x over previous
"""Trainium2 Bass kernel for nn_BraskModel (nms_detection).

Strategy
--------
The reference builds a (B,R,K,L,H) broadcast-fused tensor and applies 1-wide
linear sigmoid heads to it.  Because the heads are linear, the huge tensor
decomposes exactly:

    fts[b,r,k,l] = sigmoid( t1[b,k] + t2[b,l] + t3[b,r] + const )

with
    t1[b,k] = m[b,k] * (sk[b,k,:] @ (ws @ u) + bs @ u)        (span terms)
    t2[b,l] = X[b,l,:] @ (wx @ u + u) + bx @ u                (per-token)
    t3[b,r] = sum_l a[b,r,l] * (X[b,l,:] @ u)                 (attention)

so only the attention scores e[b,r,l] = sum_a tanh(wxa[b,l,a]+wr[r,a]+wg[b,a])*v[a]
involve heavy compute.  vb cancels in softmax.

Sharding: 8 cores = 2 groups of 4.  Cores 0-3 run the semantic attention /
forward path (fts,fte), cores 4-7 the transe attention / backward path
(bhs,bhe); within a group each core owns R/4 = 4 relations.

Perf notes: fp32 matmul streams at 4 cycles/row on TRN2; float32r streams at
1 cycle/row for N>=256, costing ~1.4e-4 relative matmul error (~13-bit
mantissa), which stays well inside tolerance after softmax amplification.
All heavy matmuls run f32r.  Small constants are packed into three blobs so
the DMA ring isn't serialized by ~25 tiny descriptors before the big loads.

Host side does only: input marshalling (X transpose), tiny weight-derived
projections (rel @ wr etc., ~5M MACs), and the inherently sequential greedy
span matcher (384-step boolean scan, whose decision margins ~3e-5 are ~100x
above fp32 rounding noise).
"""

import numpy as np

import concourse.bacc as bacc
import concourse.bass as bass
import concourse.mybir as mybir
import concourse.tile as tile
from concourse.bass_utils import run_bass_kernel_spmd

B, L, H, R, RD, A = 2, 384, 768, 16, 256, 256
K_MAX, MAX_SPAN, THR = 8, 10, 0.5
N_CORES = 8
GROUP = 4          # cores per attention group
RLOC = R // GROUP  # relations per core
BL = B * L         # 768
F32 = mybir.dt.float32
F32R = mybir.dt.float32r
F16 = mybir.dt.float16
KC = H // 128      # 6 contraction chunks

# --- CONSTF (fp32, 128 x 266): col 0 t1f | col 1 pbh | 2:10 wr | 10:138 sels
#     | 138:266 sele
CF_T1F, CF_PBH, CF_WR, CF_SELS, CF_SELE, CF_COLS = 0, 1, 2, 10, 138, 266
# --- CONSTR (f32r, 128 x 176): 0:48 v12 chunks | 48:176 vmask chunks
CR_V12, CR_VM, CR_COLS = 0, 48, 176
# --- CONSTROW (f32r, 1 x 1424): 0:512 wg | 512:528 eb | 528:1040 ob | 1040:1424 ones
CW_WG, CW_EB, CW_OB, CW_ONES, CW_COLS = 0, 512, 528, 1040, 1424

_compiled = {}


# ----------------------------------------------------------------------------
# Bass program (identical on all 8 cores; per-core behaviour is data-driven)
# ----------------------------------------------------------------------------
def _build_nc(debug=False):
    nc = bacc.Bacc("TRN2", target_bir_lowering=False, debug=debug, num_devices=1)

    xt_d = nc.dram_tensor("xt", (H, BL), F16, kind="ExternalInput")
    wx_d = nc.dram_tensor("wxm", (H, A), F16, kind="ExternalInput")
    cr_d = nc.dram_tensor("constr", (128, CR_COLS), F16, kind="ExternalInput")
    cw_d = nc.dram_tensor("constrow", (1, CW_COLS), F32R, kind="ExternalInput")
    cf_d = nc.dram_tensor("constf", (128, CF_COLS), F32, kind="ExternalInput")

    big_d = nc.dram_tensor("big", (128, 384), F32, kind="ExternalOutput")
    probs_d = nc.dram_tensor("probs", (4, BL), F32, kind="ExternalOutput")

    with tile.TileContext(nc) as tc:
        with (
            tc.tile_pool(name="const", bufs=1) as cpool,
            tc.tile_pool(name="work", bufs=1) as wpool,
            tc.tile_pool(name="zpool", bufs=3) as zpool,
            tc.tile_pool(name="ps_big", bufs=4, space="PSUM") as ps_big,
            tc.tile_pool(name="ps_small", bufs=2, space="PSUM") as ps_small,
        ):
            # ---- loads ------------------------------------------------------
            # Each dma_start costs ~0.6us of serial descriptor time on its
            # ring, so batch: XT lands as one (128, 6*768) tile via two large
            # DMAs (3 contraction chunks each, so matmuls start after half),
            # WX as one (128, 6*256) tile, spread over the sync / scalar /
            # gpsimd rings.
            xtb = cpool.tile([128, KC * BL], F16, tag="xtb")
            wxb = cpool.tile([128, KC * A], F16, tag="wxb")
            cr_sb = cpool.tile([128, CR_COLS], F16, tag="cr")
            cw_sb = cpool.tile([1, CW_COLS], F32R, tag="cw")
            cf_sb = cpool.tile([128, CF_COLS], F32, tag="cf")
            wx_r = wx_d[:].rearrange("(c p) a -> p c a", p=128)
            nc.sync.dma_start(wxb[:], wx_r)
            rings = {0: nc.sync, 1: nc.sync, 2: nc.scalar, 3: nc.scalar,
                     4: nc.gpsimd, 5: nc.gpsimd}
            nc.gpsimd.dma_start(cr_sb[:], cr_d[:])
            nc.gpsimd.dma_start(cw_sb[:], cw_d[:])
            for k in range(KC):
                rings[k].dma_start(xtb[:, k * BL:(k + 1) * BL],
                                   xt_d[k * 128:(k + 1) * 128, :])
            nc.sync.dma_start(cf_sb[:], cf_d[:])

            def xt_sl(k, lo, hi):
                return xtb[:, k * BL + lo: k * BL + hi]

            def wx_sl(k, at):
                return wxb[:, k * A + at * 128: k * A + (at + 1) * 128]

            def v12(k):
                return cr_sb[:, CR_V12 + 8 * k: CR_V12 + 8 * (k + 1)]

            def vmask(c, p):
                return cr_sb[:, CR_VM + 64 * c + 8 * p: CR_VM + 64 * c + 8 * (p + 1)]

            def wg_row(b, at):
                o = CW_WG + 256 * b + 128 * at
                return cw_sb[0:1, o:o + 128]

            def eb_row(b):
                return cw_sb[0:1, CW_EB + 8 * b: CW_EB + 8 * (b + 1)]

            def ob_row(v):
                return cw_sb[0:1, CW_OB + 128 * v: CW_OB + 128 * (v + 1)]

            ones384 = cw_sb[0:1, CW_ONES:CW_ONES + 384]

            # ---- XS = V12^T @ X^T -> (8, BL) and WXA = WX^T @ X^T (+wg) ----
            xs_ps = [ps_small.tile([8, 384], F32, tag="ps_small", name=f"xs_ps{i}")
                     for i in range(2)]
            wxa_ps = [ps_big.tile([128, 384], F32, tag="ps_big", name=f"wxa_ps{i}")
                      for i in range(4)]
            for k in range(KC):
                for at in range(2):
                    for b in range(2):
                        nc.tensor.matmul(
                            wxa_ps[at * 2 + b][:], wx_sl(k, at),
                            xt_sl(k, b * 384, (b + 1) * 384),
                            start=(k == 0), stop=False,
                        )
                for half in range(2):
                    nc.tensor.matmul(
                        xs_ps[half][:], v12(k),
                        xt_sl(k, half * 384, (half + 1) * 384),
                        start=(k == 0), stop=(k == KC - 1),
                    )
            # fold wg[b,:] into wxa (rank-1 update): removes b from tanh bias
            for at in range(2):
                for b in range(2):
                    nc.tensor.matmul(
                        wxa_ps[at * 2 + b][:], wg_row(b, at), ones384,
                        start=False, stop=True,
                    )

            wxa_sb = [wpool.tile([128, BL], F32, tag=f"wxa_sb{c}", name=f"wxa_sb{c}")
                      for c in range(2)]
            for at in range(2):
                for b in range(2):
                    nc.vector.tensor_copy(
                        wxa_sb[at][:, b * 384:(b + 1) * 384], wxa_ps[at * 2 + b][:])
            xs_sb = wpool.tile([8, BL], F32, tag="xs_sb")
            for half in range(2):
                nc.vector.tensor_copy(
                    xs_sb[:, half * 384:(half + 1) * 384], xs_ps[half][:])
            # xs_s, xs_e, t2_s, t2_e rows at base partition 0, rounded to f32r
            xsrow = [wpool.tile([1, BL], F32R, tag=f"xsrow_{i}", name=f"xsrow{i}")
                     for i in range(4)]
            for i in range(4):
                nc.gpsimd.dma_start(xsrow[i][:], xs_sb[4 + i:5 + i, :])

            # ---- probs outputs: sigmoid(x) = 0.5*(1+tanh(x/2)) -------------
            pr_t = wpool.tile([4, BL], F32, tag="pr_t")
            nc.scalar.activation(pr_t[:], xs_sb[0:4, :],
                                 mybir.ActivationFunctionType.Tanh,
                                 bias=cf_sb[0:4, CF_PBH:CF_PBH + 1], scale=0.5)
            probs_sb = wpool.tile([4, BL], F32, tag="probs_sb")
            nc.vector.tensor_scalar(probs_sb[:], pr_t[:], 1.0, 0.5,
                                    mybir.AluOpType.add, mybir.AluOpType.mult)
            nc.sync.dma_start(probs_d[:], probs_sb[:])

            # ---- z = tanh(wxa + wr[rl]) ; E[pair,l] = sum_a z*v ------------
            e_ps = ps_small.tile([8, 384], F32, tag="ps_small", name="e_ps")
            n_emm = 0
            for rl in range(RLOC):
                for c in range(2):
                    z = zpool.tile([128, BL], F16, tag="z", name="z")
                    wrcol = CF_WR + 4 * c + rl
                    nc.scalar.activation(z[:], wxa_sb[c][:],
                                         mybir.ActivationFunctionType.Tanh,
                                         bias=cf_sb[:, wrcol:wrcol + 1], scale=1.0)
                    for b in range(2):
                        p = b * RLOC + rl
                        nc.tensor.matmul(
                            e_ps[:], vmask(c, p), z[:, b * 384:(b + 1) * 384],
                            start=(n_emm == 0), stop=(n_emm == 2 * 2 * RLOC - 1),
                        )
                        n_emm += 1

            # ---- softmax over l (per pair row) -----------------------------
            exp_sb = wpool.tile([8, 384], F32, tag="exp_sb")
            esum = wpool.tile([8, 1], F32, tag="esum")
            nc.scalar.activation(exp_sb[:], e_ps[:],
                                 mybir.ActivationFunctionType.Exp,
                                 accum_out=esum[:, 0:1])
            rsum = wpool.tile([8, 1], F32, tag="rsum")
            nc.vector.reciprocal(rsum[:], esum[:])

            # ---- t3[pair] = sum_l a * xs_u (per head) ----------------------
            t3 = [wpool.tile([8, 1], F32, tag=f"t3_{hu}", name=f"t3_{hu}")
                  for hu in range(2)]
            scr = wpool.tile([8, 384], F32, tag="scr")
            for hu in range(2):
                xb_ps = ps_small.tile([8, 384], F32, tag="ps_small", name="xb_ps")
                for b in range(2):
                    nc.tensor.matmul(
                        xb_ps[:], eb_row(b),
                        xsrow[hu][0:1, b * 384:(b + 1) * 384],
                        start=(b == 0), stop=(b == 1),
                    )
                nc.vector.tensor_mul(scr[:], exp_sb[:], xb_ps[:])
                t3u = wpool.tile([8, 1], F32, tag="t3u", name="t3u", bufs=2)
                nc.vector.reduce_sum(t3u[:], scr[:], axis=mybir.AxisListType.X)
                nc.vector.tensor_scalar_mul(t3[hu][:], t3u[:], rsum[:, 0:1])

            # ---- expand t3 to 128 output rows, add t1, halve ---------------
            t3e_ps = ps_big.tile([128, 1], F32, tag="ps_big", name="t3e_ps")
            nc.tensor.matmul(t3e_ps[:], cf_sb[0:8, CF_SELS:CF_SELS + 128],
                             t3[0][:], start=True, stop=False)
            nc.tensor.matmul(t3e_ps[:], cf_sb[0:8, CF_SELE:CF_SELE + 128],
                             t3[1][:], start=False, stop=True)
            bias_sb = wpool.tile([128, 1], F32, tag="bias_sb")
            nc.vector.tensor_scalar(bias_sb[:], t3e_ps[:],
                                    cf_sb[:, CF_T1F:CF_T1F + 1], 0.5,
                                    mybir.AluOpType.add, mybir.AluOpType.mult)

            # ---- big output: rows (head,b,rl,k) x 384 ----------------------
            tb_ps = ps_big.tile([128, 384], F32, tag="ps_big", name="tb_ps")
            n_tb = 0
            for hu in range(2):
                for b in range(2):
                    nc.tensor.matmul(
                        tb_ps[:], ob_row(hu * 2 + b),
                        xsrow[2 + hu][0:1, b * 384:(b + 1) * 384],
                        start=(n_tb == 0), stop=(n_tb == 3),
                    )
                    n_tb += 1
            th_sb = wpool.tile([128, 384], F32, tag="th_sb")
            nc.scalar.activation(th_sb[:], tb_ps[:],
                                 mybir.ActivationFunctionType.Tanh,
                                 bias=bias_sb[:, 0:1], scale=0.5)
            out_sb = wpool.tile([128, 384], F32, tag="out_sb")
            nc.vector.tensor_scalar(out_sb[:], th_sb[:], 1.0, 0.5,
                                    mybir.AluOpType.add, mybir.AluOpType.mult)
            nc.sync.dma_start(big_d[:], out_sb[:])

    nc.compile()
    return nc


# ----------------------------------------------------------------------------
# Host-side pieces
# ----------------------------------------------------------------------------
def _sigmoid(x):
    return 1.0 / (1.0 + np.exp(-x))


def _extract_sk(Xb, so, eo):
    """Greedy span matcher — exact transcription of reference.extract_sk."""
    Lp = L + MAX_SPAN
    eo_pad = np.concatenate([eo, np.zeros(MAX_SPAN, bool)])
    consumed = np.zeros(Lp, bool)
    valid = np.zeros(L, bool)
    e_arr = np.zeros(L, np.int64)
    for s in range(L):
        win = eo_pad[s:s + MAX_SPAN] & ~consumed[s:s + MAX_SPAN]
        ok = bool(so[s]) and bool(win.any())
        e = s + int(np.argmax(win))
        if ok:
            consumed[e] = True
        valid[s] = ok
        e_arr[s] = e
    rank = np.cumsum(valid) - 1
    slot = np.where(valid & (rank < K_MAX), rank, K_MAX)
    span = (Xb + Xb[np.clip(e_arr, 0, L - 1)]) * np.float32(0.5)
    sk = np.zeros((K_MAX + 1, H), np.float32)
    m = np.zeros((K_MAX + 1,), np.float32)
    np.add.at(sk, slot, np.where(valid[:, None], span, np.float32(0)))
    np.add.at(m, slot, valid.astype(np.float32))
    return sk[:K_MAX], m[:K_MAX]


def kernel(description_embeddings, description_mean_embeddings, description_ids,
           semantic_relation_embeddings, transe_relation_embeddings, params):
    p = {k: np.asarray(v, dtype=np.float32) for k, v in params.items()}
    X = np.asarray(description_embeddings, np.float32)
    Xm = np.asarray(description_mean_embeddings, np.float32)
    rels = {
        'f': np.asarray(semantic_relation_embeddings, np.float32),
        'b': np.asarray(transe_relation_embeddings, np.float32),
    }
    Xf = X.reshape(BL, H)
    XT = np.ascontiguousarray(Xf.T)
    XT16 = XT.astype(np.float16)

    # probabilities for the span matcher (margins >> fp rounding noise)
    pv = {}
    for pre in ('fhp', 'btp'):
        pv[pre + '_s'] = _sigmoid(Xf @ p[pre + '_sw'][:, 0] + p[pre + '_sb'][0])
        pv[pre + '_e'] = _sigmoid(Xf @ p[pre + '_ew'][:, 0] + p[pre + '_eb'][0])

    sk = {}
    msk = {}
    for path, pre in (('f', 'fhp'), ('b', 'btp')):
        so = (pv[pre + '_s'] >= THR).reshape(B, L)
        eo = (pv[pre + '_e'] >= THR).reshape(B, L)
        s_l, m_l = [], []
        for b in range(B):
            s_, m_ = _extract_sk(X[b], so[b], eo[b])
            s_l.append(s_)
            m_l.append(m_)
        sk[path] = np.stack(s_l)
        msk[path] = np.stack(m_l)

    pidx = np.arange(128)
    b_of_p = (pidx % 64) // 32
    rl_of_p = (pidx % 32) // 8
    k_of_p = pidx % 8

    # ---- CONSTROW blob (f32r semantics, fp32 storage); core-independent ----
    constrow = np.zeros((1, CW_COLS), np.float32)
    for b in range(B):
        constrow[0, CW_EB + 8 * b + b * RLOC: CW_EB + 8 * b + (b + 1) * RLOC] = 1.0
    for hu in range(2):
        for b in range(2):
            v0 = hu * 2 + b
            o = CW_OB + 128 * v0 + hu * 64 + b * 32
            constrow[0, o:o + 32] = 1.0
    constrow[0, CW_ONES:CW_ONES + 384] = 1.0

    # per-group (attention/path) data
    group = {}
    for path, att, fuse, head in (('f', 'sra', 'fef', 'ftp'),
                                  ('b', 'tra', 'feb', 'bhp')):
        rel = rels[path]
        wr_proj = rel @ p[att + '_wr'] + p[att + '_br']          # (R, A)
        wg_proj = Xm @ p[att + '_wg'] + p[att + '_bg']           # (B, A)
        v = p[att + '_v'][:, 0]                                  # (A,)

        cw = constrow.copy()
        cw[0, CW_WG:CW_WG + 512] = wg_proj.reshape(-1)

        constr = np.zeros((128, CR_COLS), np.float32)
        for k in range(KC):
            pass  # v12 filled below
        for c in range(2):
            for pp in range(8):
                constr[:, CR_VM + 64 * c + 8 * pp + pp] = v[c * 128:(c + 1) * 128]

        t1f = np.zeros((128,), np.float32)
        v12 = np.zeros((H, 8), np.float32)
        v12[:, 0] = p['fhp_sw'][:, 0]
        v12[:, 1] = p['fhp_ew'][:, 0]
        v12[:, 2] = p['btp_sw'][:, 0]
        v12[:, 3] = p['btp_ew'][:, 0]
        for hu, suf in enumerate(('s', 'e')):
            u = p[head + '_' + suf + 'w'][:, 0]
            ub = p[head + '_' + suf + 'b'][0]
            v12[:, 4 + hu] = u
            v12[:, 6 + hu] = p[fuse + '_wx'] @ u + u
            t1 = msk[path] * (sk[path] @ (p[fuse + '_ws'] @ u)
                              + p[fuse + '_bs'] @ u)             # (B, K)
            const = p[fuse + '_bx'] @ u + ub
            rows = slice(hu * 64, (hu + 1) * 64)
            t1f[rows] = t1[b_of_p[rows], k_of_p[rows]] + const
        for k in range(KC):
            constr[:, CR_V12 + 8 * k: CR_V12 + 8 * (k + 1)] = \
                v12[k * 128:(k + 1) * 128, :]

        group[path] = dict(
            wx16=np.ascontiguousarray(p[att + '_wx']).astype(np.float16), wr_proj=wr_proj,
            constr=constr, cw=cw, t1f=t1f,
        )

    # ---- CONSTF (per core: wr slice differs) -------------------------------
    sels = np.zeros((8, 128), np.float32)
    sele = np.zeros((8, 128), np.float32)
    for pp in range(128):
        q = b_of_p[pp] * RLOC + rl_of_p[pp]
        (sels if pp < 64 else sele)[q, pp] = 1.0
    pbh = 0.5 * np.array([p['fhp_sb'][0], p['fhp_eb'][0],
                          p['btp_sb'][0], p['btp_eb'][0]], np.float32)

    in_maps = []
    for core in range(N_CORES):
        path = 'f' if core < GROUP else 'b'
        g = core % GROUP
        gd = group[path]
        constf = np.zeros((128, CF_COLS), np.float32)
        constf[:, CF_T1F] = gd['t1f']
        constf[0:4, CF_PBH] = pbh
        for c in range(2):
            for rl in range(RLOC):
                constf[:, CF_WR + 4 * c + rl] = \
                    gd['wr_proj'][g * RLOC + rl, c * 128:(c + 1) * 128]
        constf[0:8, CF_SELS:CF_SELS + 128] = sels
        constf[0:8, CF_SELE:CF_SELE + 128] = sele
        in_maps.append({
            'xt': XT16, 'wxm': gd['wx16'], 'constr': gd['constr'].astype(np.float16),
            'constrow': gd['cw'], 'constf': constf,
        })

    if 'nc' not in _compiled:
        _compiled['nc'] = _build_nc(debug=False)
    nc = _compiled['nc']
    res = run_bass_kernel_spmd(nc, in_maps, core_ids=list(range(N_CORES)))
    outs = res.results

    probs = outs[0]['probs']
    fhs = probs[0].reshape(B, L, 1).astype(np.float32)
    fhe = probs[1].reshape(B, L, 1).astype(np.float32)
    bts = probs[2].reshape(B, L, 1).astype(np.float32)
    bte = probs[3].reshape(B, L, 1).astype(np.float32)

    def gather_big(cores, row0):
        full = np.zeros((B, R, K_MAX, L, 1), np.float32)
        for g, core in enumerate(cores):
            blk = outs[core]['big'][row0:row0 + 64].reshape(2, RLOC, K_MAX, L)
            full[:, g * RLOC:(g + 1) * RLOC] = blk[..., None]
        return full

    fts = gather_big(range(0, 4), 0)
    fte = gather_big(range(0, 4), 64)
    bhs = gather_big(range(4, 8), 0)
    bhe = gather_big(range(4, 8), 64)

    return fhs, fhe, fts, fte, bts, bte, bhs, bhe


# revision 13
# speedup vs baseline: 1.0574x; 1.0313x over previous
"""Trainium2 Bass kernel for nn_BraskModel (nms_detection).

Strategy
--------
The reference builds a (B,R,K,L,H) broadcast-fused tensor and applies 1-wide
linear sigmoid heads to it.  Because the heads are linear, the huge tensor
decomposes exactly:

    fts[b,r,k,l] = sigmoid( t1[b,k] + t2[b,l] + t3[b,r] + const )

with
    t1[b,k] = m[b,k] * (sk[b,k,:] @ (ws @ u) + bs @ u)        (span terms)
    t2[b,l] = X[b,l,:] @ (wx @ u + u) + bx @ u                (per-token)
    t3[b,r] = sum_l a[b,r,l] * (X[b,l,:] @ u)                 (attention)

so only the attention scores e[b,r,l] = sum_a tanh(wxa[b,l,a]+wr[r,a]+wg[b,a])*v[a]
involve heavy compute.  vb cancels in softmax.

Sharding: 8 cores = 2 groups of 4.  Cores 0-3 run the semantic attention /
forward path (fts,fte), cores 4-7 the transe attention / backward path
(bhs,bhe); within a group each core owns R/4 = 4 relations.

Perf notes: fp32 matmul streams at 4 cycles/row on TRN2; float32r streams at
1 cycle/row for N>=256, costing ~1.4e-4 relative matmul error (~13-bit
mantissa), which stays well inside tolerance after softmax amplification.
All heavy matmuls run f32r.  Small constants are packed into three blobs so
the DMA ring isn't serialized by ~25 tiny descriptors before the big loads.

Host side does only: input marshalling (X transpose), tiny weight-derived
projections (rel @ wr etc., ~5M MACs), and the inherently sequential greedy
span matcher (384-step boolean scan, whose decision margins ~3e-5 are ~100x
above fp32 rounding noise).
"""

import numpy as np

import concourse.bacc as bacc
import concourse.bass as bass
import concourse.mybir as mybir
import concourse.tile as tile
from concourse.bass_utils import run_bass_kernel_spmd

B, L, H, R, RD, A = 2, 384, 768, 16, 256, 256
K_MAX, MAX_SPAN, THR = 8, 10, 0.5
N_CORES = 8
GROUP = 4          # cores per attention group
RLOC = R // GROUP  # relations per core
BL = B * L         # 768
F32 = mybir.dt.float32
F32R = mybir.dt.float32r
F16 = mybir.dt.float16
KC = H // 128      # 6 contraction chunks

# --- CONSTF (fp32, 128 x 266): col 0 t1f | col 1 pbh | 2:10 wr | 10:138 sels
#     | 138:266 sele
CF_T1F, CF_PBH, CF_WR, CF_SELS, CF_SELE, CF_COLS = 0, 1, 2, 10, 138, 266
# --- CONSTR (f32r, 128 x 176): 0:48 v12 chunks | 48:176 vmask chunks
CR_V12, CR_VM, CR_COLS = 0, 48, 176
# --- CONSTROW (f32r, 1 x 1424): 0:512 wg | 512:528 eb | 528:1040 ob | 1040:1424 ones
CW_WG, CW_EB, CW_OB, CW_ONES, CW_COLS = 0, 512, 528, 1040, 1424

_compiled = {}


# ----------------------------------------------------------------------------
# Bass program (identical on all 8 cores; per-core behaviour is data-driven)
# ----------------------------------------------------------------------------
def _build_nc(debug=False):
    nc = bacc.Bacc("TRN2", target_bir_lowering=False, debug=debug, num_devices=1)

    xt_d = nc.dram_tensor("xt", (H, BL), F16, kind="ExternalInput")
    wx_d = nc.dram_tensor("wxm", (H, A), F16, kind="ExternalInput")
    cr_d = nc.dram_tensor("constr", (128, CR_COLS), F16, kind="ExternalInput")
    cw_d = nc.dram_tensor("constrow", (1, CW_COLS), F32R, kind="ExternalInput")
    cf_d = nc.dram_tensor("constf", (128, CF_COLS), F32, kind="ExternalInput")

    big_d = nc.dram_tensor("big", (128, 384), F32, kind="ExternalOutput")
    probs_d = nc.dram_tensor("probs", (4, BL), F32, kind="ExternalOutput")

    with tile.TileContext(nc) as tc:
        with (
            tc.tile_pool(name="const", bufs=1) as cpool,
            tc.tile_pool(name="work", bufs=1) as wpool,
            tc.tile_pool(name="zpool", bufs=3) as zpool,
            tc.tile_pool(name="ps_big", bufs=4, space="PSUM") as ps_big,
            tc.tile_pool(name="ps_small", bufs=2, space="PSUM") as ps_small,
        ):
            # ---- loads ------------------------------------------------------
            # Each dma_start costs ~0.6us of serial descriptor time on its
            # ring, so batch: XT lands as one (128, 6*768) tile via two large
            # DMAs (3 contraction chunks each, so matmuls start after half),
            # WX as one (128, 6*256) tile, spread over the sync / scalar /
            # gpsimd rings.
            xtb = cpool.tile([128, KC * BL], F16, tag="xtb")
            wxb = cpool.tile([128, KC * A], F16, tag="wxb")
            cr_sb = cpool.tile([128, CR_COLS], F16, tag="cr")
            cw_sb = cpool.tile([1, CW_COLS], F32R, tag="cw")
            cf_sb = cpool.tile([128, CF_COLS], F32, tag="cf")
            # PE warmup: dependency-free fp32 matmuls keep the HAM clock
            # gate busy during the DMA window so the real f32r/fp16 stream
            # runs at 2.4 GHz. First real matmul waits for DMA (~13us), the
            # warmup ends ~11.5us, so it costs nothing.
            with tc.tile_pool(name="ps_warm", bufs=1, space="PSUM") as ps_warm:
                wsc = wpool.tile([128, 512], F32, tag="wsc")
                nc.vector.memset(wsc[:], 0.0)
                warm_ps = ps_warm.tile([128, 512], F32, tag="warm")
                for i in range(3):
                    nc.tensor.matmul(warm_ps[:], wsc[:, 0:128], wsc[:],
                                     start=True, stop=True)
            wx_r = wx_d[:].rearrange("(c p) a -> p c a", p=128)
            nc.sync.dma_start(wxb[:], wx_r)
            rings = {0: nc.sync, 1: nc.sync, 2: nc.scalar, 3: nc.scalar,
                     4: nc.gpsimd, 5: nc.gpsimd}
            nc.gpsimd.dma_start(cr_sb[:], cr_d[:])
            nc.gpsimd.dma_start(cw_sb[:], cw_d[:])
            for k in range(KC):
                rings[k].dma_start(xtb[:, k * BL:(k + 1) * BL],
                                   xt_d[k * 128:(k + 1) * 128, :])
            nc.sync.dma_start(cf_sb[:], cf_d[:])

            def xt_sl(k, lo, hi):
                return xtb[:, k * BL + lo: k * BL + hi]

            def wx_sl(k, at):
                return wxb[:, k * A + at * 128: k * A + (at + 1) * 128]

            def v12(k):
                return cr_sb[:, CR_V12 + 8 * k: CR_V12 + 8 * (k + 1)]

            def vmask(c, p):
                return cr_sb[:, CR_VM + 64 * c + 8 * p: CR_VM + 64 * c + 8 * (p + 1)]

            def wg_row(b, at):
                o = CW_WG + 256 * b + 128 * at
                return cw_sb[0:1, o:o + 128]

            def eb_row(b):
                return cw_sb[0:1, CW_EB + 8 * b: CW_EB + 8 * (b + 1)]

            def ob_row(v):
                return cw_sb[0:1, CW_OB + 128 * v: CW_OB + 128 * (v + 1)]

            ones384 = cw_sb[0:1, CW_ONES:CW_ONES + 384]

            # ---- XS = V12^T @ X^T -> (8, BL) and WXA = WX^T @ X^T (+wg) ----
            xs_ps = [ps_small.tile([8, 384], F32, tag="ps_small", name=f"xs_ps{i}")
                     for i in range(2)]
            wxa_ps = [ps_big.tile([128, 384], F32, tag="ps_big", name=f"wxa_ps{i}")
                      for i in range(4)]
            for k in range(KC):
                for at in range(2):
                    for b in range(2):
                        nc.tensor.matmul(
                            wxa_ps[at * 2 + b][:], wx_sl(k, at),
                            xt_sl(k, b * 384, (b + 1) * 384),
                            start=(k == 0), stop=False,
                        )
                for half in range(2):
                    nc.tensor.matmul(
                        xs_ps[half][:], v12(k),
                        xt_sl(k, half * 384, (half + 1) * 384),
                        start=(k == 0), stop=(k == KC - 1),
                    )
            # fold wg[b,:] into wxa (rank-1 update): removes b from tanh bias
            for at in range(2):
                for b in range(2):
                    nc.tensor.matmul(
                        wxa_ps[at * 2 + b][:], wg_row(b, at), ones384,
                        start=False, stop=True,
                    )

            wxa_sb = [wpool.tile([128, BL], F32, tag=f"wxa_sb{c}", name=f"wxa_sb{c}")
                      for c in range(2)]
            for at in range(2):
                for b in range(2):
                    nc.vector.tensor_copy(
                        wxa_sb[at][:, b * 384:(b + 1) * 384], wxa_ps[at * 2 + b][:])
            xs_sb = wpool.tile([8, BL], F32, tag="xs_sb")
            for half in range(2):
                nc.vector.tensor_copy(
                    xs_sb[:, half * 384:(half + 1) * 384], xs_ps[half][:])
            # xs_s, xs_e, t2_s, t2_e rows at base partition 0, rounded to f32r
            xsrow = [wpool.tile([1, BL], F32R, tag=f"xsrow_{i}", name=f"xsrow{i}")
                     for i in range(4)]
            for i in range(4):
                nc.gpsimd.dma_start(xsrow[i][:], xs_sb[4 + i:5 + i, :])

            # ---- probs outputs: sigmoid(x) = 0.5*(1+tanh(x/2)) -------------
            pr_t = wpool.tile([4, BL], F32, tag="pr_t")
            nc.scalar.activation(pr_t[:], xs_sb[0:4, :],
                                 mybir.ActivationFunctionType.Tanh,
                                 bias=cf_sb[0:4, CF_PBH:CF_PBH + 1], scale=0.5)
            probs_sb = wpool.tile([4, BL], F32, tag="probs_sb")
            nc.vector.tensor_scalar(probs_sb[:], pr_t[:], 1.0, 0.5,
                                    mybir.AluOpType.add, mybir.AluOpType.mult)
            nc.sync.dma_start(probs_d[:], probs_sb[:])

            # ---- z = tanh(wxa + wr[rl]) ; E[pair,l] = sum_a z*v ------------
            e_ps = ps_small.tile([8, 384], F32, tag="ps_small", name="e_ps")
            n_emm = 0
            for rl in range(RLOC):
                for c in range(2):
                    z = zpool.tile([128, BL], F16, tag="z", name="z")
                    wrcol = CF_WR + 4 * c + rl
                    nc.scalar.activation(z[:], wxa_sb[c][:],
                                         mybir.ActivationFunctionType.Tanh,
                                         bias=cf_sb[:, wrcol:wrcol + 1], scale=1.0)
                    for b in range(2):
                        p = b * RLOC + rl
                        nc.tensor.matmul(
                            e_ps[:], vmask(c, p), z[:, b * 384:(b + 1) * 384],
                            start=(n_emm == 0), stop=(n_emm == 2 * 2 * RLOC - 1),
                        )
                        n_emm += 1

            # ---- softmax over l (per pair row) -----------------------------
            exp_sb = wpool.tile([8, 384], F32, tag="exp_sb")
            esum = wpool.tile([8, 1], F32, tag="esum")
            nc.scalar.activation(exp_sb[:], e_ps[:],
                                 mybir.ActivationFunctionType.Exp,
                                 accum_out=esum[:, 0:1])
            rsum = wpool.tile([8, 1], F32, tag="rsum")
            nc.vector.reciprocal(rsum[:], esum[:])

            # ---- t3[pair] = sum_l a * xs_u (per head) ----------------------
            t3 = [wpool.tile([8, 1], F32, tag=f"t3_{hu}", name=f"t3_{hu}")
                  for hu in range(2)]
            scr = wpool.tile([8, 384], F32, tag="scr")
            for hu in range(2):
                xb_ps = ps_small.tile([8, 384], F32, tag="ps_small", name="xb_ps")
                for b in range(2):
                    nc.tensor.matmul(
                        xb_ps[:], eb_row(b),
                        xsrow[hu][0:1, b * 384:(b + 1) * 384],
                        start=(b == 0), stop=(b == 1),
                    )
                nc.vector.tensor_mul(scr[:], exp_sb[:], xb_ps[:])
                t3u = wpool.tile([8, 1], F32, tag="t3u", name="t3u", bufs=2)
                nc.vector.reduce_sum(t3u[:], scr[:], axis=mybir.AxisListType.X)
                nc.vector.tensor_scalar_mul(t3[hu][:], t3u[:], rsum[:, 0:1])

            # ---- expand t3 to 128 output rows, add t1, halve ---------------
            t3e_ps = ps_big.tile([128, 1], F32, tag="ps_big", name="t3e_ps")
            nc.tensor.matmul(t3e_ps[:], cf_sb[0:8, CF_SELS:CF_SELS + 128],
                             t3[0][:], start=True, stop=False)
            nc.tensor.matmul(t3e_ps[:], cf_sb[0:8, CF_SELE:CF_SELE + 128],
                             t3[1][:], start=False, stop=True)
            bias_sb = wpool.tile([128, 1], F32, tag="bias_sb")
            nc.vector.tensor_scalar(bias_sb[:], t3e_ps[:],
                                    cf_sb[:, CF_T1F:CF_T1F + 1], 0.5,
                                    mybir.AluOpType.add, mybir.AluOpType.mult)

            # ---- big output: rows (head,b,rl,k) x 384 ----------------------
            tb_ps = ps_big.tile([128, 384], F32, tag="ps_big", name="tb_ps")
            n_tb = 0
            for hu in range(2):
                for b in range(2):
                    nc.tensor.matmul(
                        tb_ps[:], ob_row(hu * 2 + b),
                        xsrow[2 + hu][0:1, b * 384:(b + 1) * 384],
                        start=(n_tb == 0), stop=(n_tb == 3),
                    )
                    n_tb += 1
            th_sb = wpool.tile([128, 384], F32, tag="th_sb")
            nc.scalar.activation(th_sb[:], tb_ps[:],
                                 mybir.ActivationFunctionType.Tanh,
                                 bias=bias_sb[:, 0:1], scale=0.5)
            out_sb = wpool.tile([128, 384], F32, tag="out_sb")
            nc.vector.tensor_scalar(out_sb[:], th_sb[:], 1.0, 0.5,
                                    mybir.AluOpType.add, mybir.AluOpType.mult)
            nc.sync.dma_start(big_d[:], out_sb[:])

    nc.compile()
    return nc


# ----------------------------------------------------------------------------
# Host-side pieces
# ----------------------------------------------------------------------------
def _sigmoid(x):
    return 1.0 / (1.0 + np.exp(-x))


def _extract_sk(Xb, so, eo):
    """Greedy span matcher — exact transcription of reference.extract_sk."""
    Lp = L + MAX_SPAN
    eo_pad = np.concatenate([eo, np.zeros(MAX_SPAN, bool)])
    consumed = np.zeros(Lp, bool)
    valid = np.zeros(L, bool)
    e_arr = np.zeros(L, np.int64)
    for s in range(L):
        win = eo_pad[s:s + MAX_SPAN] & ~consumed[s:s + MAX_SPAN]
        ok = bool(so[s]) and bool(win.any())
        e = s + int(np.argmax(win))
        if ok:
            consumed[e] = True
        valid[s] = ok
        e_arr[s] = e
    rank = np.cumsum(valid) - 1
    slot = np.where(valid & (rank < K_MAX), rank, K_MAX)
    span = (Xb + Xb[np.clip(e_arr, 0, L - 1)]) * np.float32(0.5)
    sk = np.zeros((K_MAX + 1, H), np.float32)
    m = np.zeros((K_MAX + 1,), np.float32)
    np.add.at(sk, slot, np.where(valid[:, None], span, np.float32(0)))
    np.add.at(m, slot, valid.astype(np.float32))
    return sk[:K_MAX], m[:K_MAX]


def kernel(description_embeddings, description_mean_embeddings, description_ids,
           semantic_relation_embeddings, transe_relation_embeddings, params):
    p = {k: np.asarray(v, dtype=np.float32) for k, v in params.items()}
    X = np.asarray(description_embeddings, np.float32)
    Xm = np.asarray(description_mean_embeddings, np.float32)
    rels = {
        'f': np.asarray(semantic_relation_embeddings, np.float32),
        'b': np.asarray(transe_relation_embeddings, np.float32),
    }
    Xf = X.reshape(BL, H)
    XT = np.ascontiguousarray(Xf.T)
    XT16 = XT.astype(np.float16)

    # probabilities for the span matcher (margins >> fp rounding noise)
    pv = {}
    for pre in ('fhp', 'btp'):
        pv[pre + '_s'] = _sigmoid(Xf @ p[pre + '_sw'][:, 0] + p[pre + '_sb'][0])
        pv[pre + '_e'] = _sigmoid(Xf @ p[pre + '_ew'][:, 0] + p[pre + '_eb'][0])

    sk = {}
    msk = {}
    for path, pre in (('f', 'fhp'), ('b', 'btp')):
        so = (pv[pre + '_s'] >= THR).reshape(B, L)
        eo = (pv[pre + '_e'] >= THR).reshape(B, L)
        s_l, m_l = [], []
        for b in range(B):
            s_, m_ = _extract_sk(X[b], so[b], eo[b])
            s_l.append(s_)
            m_l.append(m_)
        sk[path] = np.stack(s_l)
        msk[path] = np.stack(m_l)

    pidx = np.arange(128)
    b_of_p = (pidx % 64) // 32
    rl_of_p = (pidx % 32) // 8
    k_of_p = pidx % 8

    # ---- CONSTROW blob (f32r semantics, fp32 storage); core-independent ----
    constrow = np.zeros((1, CW_COLS), np.float32)
    for b in range(B):
        constrow[0, CW_EB + 8 * b + b * RLOC: CW_EB + 8 * b + (b + 1) * RLOC] = 1.0
    for hu in range(2):
        for b in range(2):
            v0 = hu * 2 + b
            o = CW_OB + 128 * v0 + hu * 64 + b * 32
            constrow[0, o:o + 32] = 1.0
    constrow[0, CW_ONES:CW_ONES + 384] = 1.0

    # per-group (attention/path) data
    group = {}
    for path, att, fuse, head in (('f', 'sra', 'fef', 'ftp'),
                                  ('b', 'tra', 'feb', 'bhp')):
        rel = rels[path]
        wr_proj = rel @ p[att + '_wr'] + p[att + '_br']          # (R, A)
        wg_proj = Xm @ p[att + '_wg'] + p[att + '_bg']           # (B, A)
        v = p[att + '_v'][:, 0]                                  # (A,)

        cw = constrow.copy()
        cw[0, CW_WG:CW_WG + 512] = wg_proj.reshape(-1)

        constr = np.zeros((128, CR_COLS), np.float32)
        for k in range(KC):
            pass  # v12 filled below
        for c in range(2):
            for pp in range(8):
                constr[:, CR_VM + 64 * c + 8 * pp + pp] = v[c * 128:(c + 1) * 128]

        t1f = np.zeros((128,), np.float32)
        v12 = np.zeros((H, 8), np.float32)
        v12[:, 0] = p['fhp_sw'][:, 0]
        v12[:, 1] = p['fhp_ew'][:, 0]
        v12[:, 2] = p['btp_sw'][:, 0]
        v12[:, 3] = p['btp_ew'][:, 0]
        for hu, suf in enumerate(('s', 'e')):
            u = p[head + '_' + suf + 'w'][:, 0]
            ub = p[head + '_' + suf + 'b'][0]
            v12[:, 4 + hu] = u
            v12[:, 6 + hu] = p[fuse + '_wx'] @ u + u
            t1 = msk[path] * (sk[path] @ (p[fuse + '_ws'] @ u)
                              + p[fuse + '_bs'] @ u)             # (B, K)
            const = p[fuse + '_bx'] @ u + ub
            rows = slice(hu * 64, (hu + 1) * 64)
            t1f[rows] = t1[b_of_p[rows], k_of_p[rows]] + const
        for k in range(KC):
            constr[:, CR_V12 + 8 * k: CR_V12 + 8 * (k + 1)] = \
                v12[k * 128:(k + 1) * 128, :]

        group[path] = dict(
            wx16=np.ascontiguousarray(p[att + '_wx']).astype(np.float16), wr_proj=wr_proj,
            constr=constr, cw=cw, t1f=t1f,
        )

    # ---- CONSTF (per core: wr slice differs) -------------------------------
    sels = np.zeros((8, 128), np.float32)
    sele = np.zeros((8, 128), np.float32)
    for pp in range(128):
        q = b_of_p[pp] * RLOC + rl_of_p[pp]
        (sels if pp < 64 else sele)[q, pp] = 1.0
    pbh = 0.5 * np.array([p['fhp_sb'][0], p['fhp_eb'][0],
                          p['btp_sb'][0], p['btp_eb'][0]], np.float32)

    in_maps = []
    for core in range(N_CORES):
        path = 'f' if core < GROUP else 'b'
        g = core % GROUP
        gd = group[path]
        constf = np.zeros((128, CF_COLS), np.float32)
        constf[:, CF_T1F] = gd['t1f']
        constf[0:4, CF_PBH] = pbh
        for c in range(2):
            for rl in range(RLOC):
                constf[:, CF_WR + 4 * c + rl] = \
                    gd['wr_proj'][g * RLOC + rl, c * 128:(c + 1) * 128]
        constf[0:8, CF_SELS:CF_SELS + 128] = sels
        constf[0:8, CF_SELE:CF_SELE + 128] = sele
        in_maps.append({
            'xt': XT16, 'wxm': gd['wx16'], 'constr': gd['constr'].astype(np.float16),
            'constrow': gd['cw'], 'constf': constf,
        })

    if 'nc' not in _compiled:
        _compiled['nc'] = _build_nc(debug=False)
    nc = _compiled['nc']
    res = run_bass_kernel_spmd(nc, in_maps, core_ids=list(range(N_CORES)))
    outs = res.results

    probs = outs[0]['probs']
    fhs = probs[0].reshape(B, L, 1).astype(np.float32)
    fhe = probs[1].reshape(B, L, 1).astype(np.float32)
    bts = probs[2].reshape(B, L, 1).astype(np.float32)
    bte = probs[3].reshape(B, L, 1).astype(np.float32)

    def gather_big(cores, row0):
        full = np.zeros((B, R, K_MAX, L, 1), np.float32)
        for g, core in enumerate(cores):
            blk = outs[core]['big'][row0:row0 + 64].reshape(2, RLOC, K_MAX, L)
            full[:, g * RLOC:(g + 1) * RLOC] = blk[..., None]
        return full

    fts = gather_big(range(0, 4), 0)
    fte = gather_big(range(0, 4), 64)
    bhs = gather_big(range(4, 8), 0)
    bhe = gather_big(range(4, 8), 64)

    return fhs, fhe, fts, fte, bts, bte, bhs, bhe
